# revision 47
# baseline (speedup 1.0000x reference)
"""Multi-head attention (B=4, N=2048, C=1024, H=16) on 8 TRN2 NeuronCores.

Sharding: (batch, query-half) grid -> 8 cores, zero collectives.
Core c handles batch b = c//2, query chunk s = c%2 (1024 queries).
Each core computes K/V for all 2048 tokens of its batch (duplicated across
the 2 cores of a batch), attention for its 1024 queries x all 16 heads, and
the output projection for its query chunk. Outputs are disjoint slices of y.

Token-roll trick: the host passes x^T with token columns rolled so that the
core's own query half is always columns [0, 1024) -> identical SPMD graph on
all cores. Softmax/AV are permutation-invariant in key order, so the rolled
key order does not change results.

v2 schedule: the ACT engine's exp is a hard ~294us floor (256 tiles x
(1024+352)cyc @ 1.2GHz), so the whole kernel is organized as a software
pipeline that never lets ACT wait:
  - queries processed in two sequential 512-halves per head pair, so the
    AV accumulators shrink to 1 PSUM bank each; PSUM map = score tiles
    sta/stb (2 banks each, per-head, double-buffered against their exp) +
    av accumulators (2 banks) + a 2-deep quantum ring (2 banks) = 8.
  - per key-tile-pair window: S_a(n+1) is emitted before AV_a(n)/AV_b(n)
    so exp_a(n+1) can start the moment exp_b(n) retires.
  - qkv/proj GEMM quanta (contract-1024, full-array) are drip-fed into the
    PE slack of each window through the psq ring at ~3 matmuls/window.
Softmax denominator rides the AV matmul via a ones-column widening of V
(AV row 64 = sum_k P); normalize broadcasts the denominator to 64
partitions with a contract-1 ones matmul FIRST, then takes the reciprocal
on all 64 partitions (the old 1-partition reciprocal was 6.5us on DVE).

Engine discipline: PE/ACT/DVE + nc.sync/nc.scalar DMAs only.
"""

import sys

for _p in ("/opt/trn_rl_repo",):
    if _p not in sys.path:
        sys.path.insert(0, _p)

import numpy as np
import ml_dtypes

import bass_rust
import concourse.bass as bass
import concourse.mybir as mybir
import concourse.tile as tile
from concourse.bass_utils import run_bass_kernel_spmd
from concourse.vector_clock import ScopedClock


# --- tail-drain wait splitting -------------------------------------------
# Walrus codegen (CoreV3GenImpl setupSyncWait) rejects CTRL-class
# instructions carrying more than a few sync waits; Tile's kernel-tail drain
# waits on every engine/DMA-queue proc used, which fails codegen.
# Split the waits across the drain plus follow-up sync-engine NOPs emitted
# before the end-of-kernel barrier — semantically identical.
_WAIT_CHUNK = 1


def _split_drain_and_barrier(self, tick_clock, wait_clock):
    drain_inst = self.nc.sync.drain()
    wait_clock.add_sem_waits(
        drain_inst.ins, ScopedClock({None: tick_clock.global_clock})
    )
    si = drain_inst.ins.sync_info
    waits = list(si.on_wait) if si is not None and si.on_wait else []
    if len(waits) > _WAIT_CHUNK:
        si.on_wait = waits[:_WAIT_CHUNK]
        rest = waits[_WAIT_CHUNK:]
        while rest:
            take, rest = rest[:_WAIT_CHUNK], rest[_WAIT_CHUNK:]
            nop = self.nc.sync.nop(nofuse=True, hint="drain_split")
            nop.ins.sync_info = bass_rust.SyncInfo(on_wait=take, on_update=[])
    self.nc.all_engine_barrier()
    popped = self.nc._tile_sem_poison_stack.pop()
    assert popped is self._sem_poison
    self.nc.clear_and_free_semaphores(list(self.sems.allocated().values()))
    self.nc.all_engine_barrier()


tile.TileContext._drain_and_barrier = _split_drain_and_barrier


def _split_multi_waits(nc, limit=1):
    """Hoist all but `limit` sync waits of every instruction onto preceding
    same-engine NOPs (this walrus rejects >1 wait on any instruction)."""
    n = 0
    for f in nc.m.functions:
        for bb in f.blocks:
            new_insts = []
            for ins in bb.instructions:
                si = ins.sync_info
                waits = list(si.on_wait) if si is not None and si.on_wait else []
                if len(waits) > limit and ins.engine not in (
                    None,
                    mybir.EngineType.Unassigned,
                ):
                    for w in waits[:-limit]:
                        nop = mybir.InstNoOp(
                            name=f"{ins.name}.wsplit{n}", ins=[], outs=[]
                        )
                        n += 1
                        nop.engine = ins.engine
                        nop.sync_info = bass_rust.SyncInfo(
                            on_wait=[w], on_update=[]
                        )
                        nc.register_instruction(nop, overwrite=True)
                        new_insts.append(nop)
                    si.on_wait = waits[-limit:]
                new_insts.append(ins)
            bb.instructions = new_insts
    return n


BF16 = mybir.dt.bfloat16
F32 = mybir.dt.float32
NPBF16 = ml_dtypes.bfloat16

B, N, C = 4, 2048, 1024
H, HD = 16, 64
SCALE = HD**-0.5
NQ = N // 2          # local queries per core
P = 128              # partitions
CCH = C // P         # 8 contract chunks
NKT = N // P         # 16 key tiles
HP = H // 2          # 8 head pairs
QG = 512             # matmul free-dim group / query half
KTP = NKT // 2       # 8 key-tile pairs per window loop
VW = HD + 1          # V widened with ones column

_CACHE = {}


def _build():
    nc = bass.Bass()

    xT_d = nc.declare_dram_parameter("xT", [C, N], BF16, isOutput=False)
    wqkT_d = nc.declare_dram_parameter(
        "wqkT", [2 * CCH, P, CCH, P], BF16, isOutput=False
    )  # host pre-packed: [row-group, c-part, c-chunk, row]
    wvT_d = nc.declare_dram_parameter("wvT", [C, C], BF16, isOutput=False)
    wpT_d = nc.declare_dram_parameter("wpT", [C, C], BF16, isOutput=False)
    bias_d = nc.declare_dram_parameter("bias", [1, C], BF16, isOutput=False)
    y_d = nc.declare_dram_parameter("y", [NQ, C], F32, isOutput=True)

    xT_v = xT_d[:].rearrange("(cc p) n -> cc p n", p=P)        # [8,128,2048]
    wvT_v = wvT_d[:].rearrange("(cc p) r -> cc p r", p=P)      # [8,128,1024]
    wpT_v = wpT_d[:].rearrange("(cc p) r -> cc p r", p=P)      # [8,128,1024]

    with tile.TileContext(nc) as tc:
        with (
            tc.tile_pool(name="big", bufs=1) as big,
            tc.tile_pool(name="consts", bufs=1) as consts,
            tc.tile_pool(name="wstream", bufs=3) as wstream,
            tc.tile_pool(name="ptpool", bufs=3) as ptpool,
            tc.tile_pool(name="stashp", bufs=4) as stashp,
            tc.tile_pool(name="recpool", bufs=2) as recpool,
            tc.tile_pool(name="ypool", bufs=2) as ypool,
            tc.tile_pool(name="ps", bufs=1, space="PSUM") as ps,
        ):
            # ---- resident SBUF tensors ----
            xT_sb = big.tile([P, CCH, N], BF16, tag="xT")
            qT_sb = big.tile([P, CCH, NQ], BF16, tag="qT")
            kT_sb = big.tile([P, CCH, N], BF16, tag="kT")
            v_sb = big.tile([P, NKT, H * VW], BF16, tag="v")
            oT_sb = big.tile([P, CCH, NQ], BF16, tag="oT")
            wpT_sb = big.tile([P, CCH, C], BF16, tag="wpT")
            wv_sb = big.tile([P, CCH, C], BF16, tag="wv")
            bias_sb = consts.tile([1, C], BF16, tag="bias")
            ones_sb = consts.tile([P, P], BF16, tag="ones")

            junk_sb = consts.tile([P, QG], BF16, tag="junk")
            bias_bc = consts.tile([P, C], F32, tag="biasbc")

            nc.vector.memset(ones_sb[:], 1.0)
            nc.vector.memset(junk_sb[:], 0.0)
            v_ones = v_sb[:].rearrange("p t (h e) -> p t h e", e=VW)[
                :, :, :, HD : HD + 1
            ]
            nc.vector.memset(v_ones, 1.0)

            # ---- startup DMAs, gating-order aware ----
            # first window needs: wqk slabs 0/8, x^T tokens [0,512) (feeds both
            # Q0-tg0 and K0-tg0 thanks to the token roll), wv head-group 0.
            # Ordered so the Q0-tg0 gate (slab_q0 + x chunks) is at the very
            # front of the queue; slab_k0 only gates K0, which runs after Q0.
            slab_q0 = wstream.tile([P, CCH, P], BF16, tag="wqk", name="slab_q0")
            slab_k0 = wstream.tile([P, CCH, P], BF16, tag="wqk", name="slab_k0")
            nc.sync.dma_start(slab_q0[:], wqkT_d[0])
            nc.sync.dma_start(bias_sb[:], bias_d[:])
            for cc in range(4):
                nc.sync.dma_start(xT_sb[:, cc, 0:QG], xT_v[cc][:, 0:QG])
            # K0 starts ~8 matmuls after Q0; its slab lands mid-x-stream
            nc.sync.dma_start(slab_k0[:], wqkT_d[CCH])
            for cc in range(4, CCH):
                nc.sync.dma_start(xT_sb[:, cc, 0:QG], xT_v[cc][:, 0:QG])
            for cc in range(CCH):
                nc.sync.dma_start(xT_sb[:, cc, QG:NQ], xT_v[cc][:, QG:NQ])
            # ACT-frontend queues: wv head-group 0 first (gates V prologue),
            # then the key-half of x^T (first needed at window ktp=3), then
            # wv head-group 1 (first needed at pair 1).
            for cc in range(CCH):
                nc.scalar.dma_start(wv_sb[:, cc, 0:QG], wvT_v[cc][:, 0:QG])
            for cc in range(CCH):
                nc.scalar.dma_start(xT_sb[:, cc, NQ:N], xT_v[cc][:, NQ:N])
            for cc in range(CCH):
                nc.scalar.dma_start(wv_sb[:, cc, QG:C], wvT_v[cc][:, QG:C])

            # ---- qkv production quanta (step-granular for interleaving) ----
            def load_qk_slab(rg):
                """DMA 128 rows of w_qk^T (columns rg*128..) as [c-part, cc, row]."""
                wslab = wstream.tile([P, CCH, P], BF16, tag="wqk")
                nc.sync.dma_start(wslab[:], wqkT_d[rg])
                return wslab

            def qk_quantum_steps(rg, wslab, tg):
                """Steps (one matmul each + final copy) producing 128 rows x
                512 tokens of Q^T or K^T through the psq ring."""
                state = {}

                def mk(cc):
                    def step():
                        if cc == 0:
                            state["q"] = ps.tile(
                                [P, QG], F32, tag="q", bufs=2, name=f"qps_{rg}_{tg}"
                            )
                        nc.tensor.matmul(
                            state["q"][:],
                            lhsT=wslab[:, cc, :],
                            rhs=xT_sb[:, cc, tg * QG : (tg + 1) * QG],
                            start=(cc == 0),
                            stop=(cc == CCH - 1),
                        )
                    return step

                def copy_step():
                    dst = qT_sb if rg < CCH else kT_sb
                    ch = rg % CCH
                    nc.vector.tensor_copy(
                        dst[:, ch, tg * QG : (tg + 1) * QG], state["q"][:]
                    )

                return [mk(cc) for cc in range(CCH)] + [copy_step]

            def v_quantum_steps(tc_i, vg):
                """Steps producing V rows for token tile tc_i, head group vg."""
                state = {}

                def mk(cc):
                    def step():
                        if cc == 0:
                            state["q"] = ps.tile(
                                [P, QG], F32, tag="q", bufs=2, name=f"vps_{tc_i}_{vg}"
                            )
                        nc.tensor.matmul(
                            state["q"][:],
                            lhsT=xT_sb[:, cc, tc_i * P : (tc_i + 1) * P],
                            rhs=wv_sb[:, cc, vg * QG : (vg + 1) * QG],
                            start=(cc == 0),
                            stop=(cc == CCH - 1),
                        )
                    return step

                def copy_step():
                    dst = v_sb[
                        :, tc_i, vg * 8 * VW : (vg + 1) * 8 * VW
                    ].rearrange("p (h e) -> p h e", e=VW)[:, :, 0:HD]
                    nc.vector.tensor_copy(
                        dst, state["q"][:].rearrange("p (h e) -> p h e", e=HD)
                    )

                return [mk(cc) for cc in range(CCH)] + [copy_step]

            def run_all(steps):
                for s in steps:
                    s()

            # ---- HAM pre-warm + bias broadcast during the DMA gate ----
            # the PE sits idle ~15us waiting for the first x/w transfers;
            # junk matmuls keep the HAM activity window busy so the real
            # prologue starts at 2.4GHz instead of the 1.2GHz cold clock;
            # sized to bridge all the way to the first DMA-gated matmul
            # (~8us) so the PE never idles into a re-throttle
            for w in range(30):
                jps = ps.tile([P, QG], F32, tag="q", bufs=2, name=f"warm_{w}")
                nc.tensor.matmul(
                    jps[:], lhsT=ones_sb[:], rhs=junk_sb[:],
                    start=True, stop=True,
                )
            # ---- prologue: only what gates window 0 ----
            # qh0 uses q tokens [0,512) (tg0) and key tiles in ktp order, so
            # Q0-tg1 / K0-tg1..3 are drip-fed JIT through the early windows.
            # v0/v1 run before K0-tg1: their inputs (wv, scalar queue) land
            # ~10us before the last x chunks K0-tg1 needs (sync queue tail).
            run_all(qk_quantum_steps(0, slab_q0, 0))
            run_all(qk_quantum_steps(CCH, slab_k0, 0))
            run_all(v_quantum_steps(0, 0))
            run_all(v_quantum_steps(1, 0))
            run_all(qk_quantum_steps(CCH, slab_k0, 1))
            # bias broadcast to all 128 partitions once; proj adds it via
            # DVE in the y copy instead of a per-group ones matmul
            for og in range(C // QG):
                bps = ps.tile([P, QG], F32, tag="q", bufs=2, name=f"bbc_{og}")
                nc.tensor.matmul(
                    bps[:],
                    lhsT=ones_sb[0:1, 0:P],
                    rhs=bias_sb[0:1, og * QG : (og + 1) * QG],
                    start=True,
                    stop=True,
                )
                nc.vector.tensor_copy(bias_bc[:, og * QG : (og + 1) * QG], bps[:])
            # K0-tg2 copy must be emitted before window 3's score emission,
            # tg3 before window 5's; at >=4 pumped steps/window both clear
            # their deadlines with >= one full window of margin.
            pair0_pre = (
                qk_quantum_steps(CCH, slab_k0, 2)
                + qk_quantum_steps(CCH, slab_k0, 3)
                + qk_quantum_steps(0, slab_q0, 1)
            )

            # ---- normalize helper (deferred off the critical path) ----
            def normalize(h, qh, stash):
                """oT rows of head h, query half qh = stash[0:64] / stash[64].

                Broadcast the denominator row to 64 partitions via a
                contract-1 ones matmul, cast it off PSUM immediately (the
                slot must not be held for the reciprocal's duration), then
                reciprocal_approx_fast (plain DVE reciprocal is ~8ns per
                free element — 4us per call) and multiply."""
                tagname = f"nrm_{h}_{qh}"
                bc = ps.tile([HD, QG], F32, tag="q", bufs=2, name=f"bc_{tagname}")
                nc.tensor.matmul(
                    bc[:],
                    lhsT=ones_sb[HD : HD + 1, 0:HD],
                    rhs=stash[HD : HD + 1, :],
                    start=True,
                    stop=True,
                )
                # 1/d via bit-trick seed + one Newton step (plain DVE
                # reciprocal is ~8ns/free-element = 4us per call; hw has no
                # tensor-tensor divide). seed = K - bits(d) done as
                # (bits(d) xor -1) + (K+1); one Newton -> ~0.2% < bf16 lsb.
                I32 = mybir.dt.int32
                rneg = recpool.tile([HD, QG], F32, tag="rneg", name=f"rn_{tagname}")
                nc.vector.tensor_scalar(
                    rneg[:].bitcast(I32), bc[:].bitcast(I32), -1, None,
                    mybir.AluOpType.bitwise_xor,
                )
                r0 = recpool.tile([HD, QG], F32, tag="r0", name=f"r0_{tagname}")
                nc.vector.tensor_scalar(
                    r0[:].bitcast(I32), rneg[:].bitcast(I32), 0x7EF311C4, None,
                    mybir.AluOpType.add,
                )
                t = recpool.tile([HD, QG], F32, tag="tt", name=f"t_{tagname}")
                nc.vector.tensor_mul(t[:], bc[:], r0[:])
                r1p = recpool.tile([HD, QG], F32, tag="r1p", name=f"r1p_{tagname}")
                nc.vector.scalar_tensor_tensor(
                    r1p[:], t[:], 2.0, r0[:],
                    mybir.AluOpType.subtract, mybir.AluOpType.mult,
                )
                base = (h % 2) * HD
                nc.vector.scalar_tensor_tensor(
                    oT_sb[base : base + HD, h // 2, qh * QG : (qh + 1) * QG],
                    stash[0:HD, :], -1.0, r1p[:],
                    mybir.AluOpType.mult, mybir.AluOpType.mult,
                )

            # vg1 V-tile schedule: first consumed at pair 4; spread over 1-3
            vg1_sched = {1: range(0, 6), 2: range(6, 12), 3: range(12, 16)}

            # ---- output projection step factory (for pair-7 overlap) ----
            def proj_steps(tc_i, og, tag="q"):
                state = {}

                def mk(cc):
                    def step():
                        if cc == 0:
                            state["q"] = ps.tile(
                                [P, QG], F32, tag=tag,
                                bufs=2 if tag == "q" else None,
                                name=f"pj_{tc_i}_{og}",
                            )
                        nc.tensor.matmul(
                            state["q"][:],
                            lhsT=oT_sb[:, cc, tc_i * P : (tc_i + 1) * P],
                            rhs=wpT_sb[:, cc, og * QG : (og + 1) * QG],
                            start=(cc == 0),
                            stop=(cc == CCH - 1),
                        )
                    return step

                def out_step():
                    y_sb = ypool.tile([P, QG], F32, tag="ysb", name=f"y_{tc_i}_{og}")
                    nc.vector.tensor_add(
                        y_sb[:], state["q"][:], bias_bc[:, og * QG : (og + 1) * QG]
                    )
                    eng = nc.sync if (tc_i + og) % 2 == 0 else nc.scalar
                    eng.dma_start(
                        y_d[tc_i * P : (tc_i + 1) * P, og * QG : (og + 1) * QG],
                        y_sb[:],
                    )

                return [mk(cc) for cc in range(CCH)] + [out_step]

            deferred = []   # pending (h, qh, stash) normalizes
            for hp in range(HP):
                ha, hb = 2 * hp, 2 * hp + 1

                pending = pair0_pre if hp == 0 else []
                if hp + 1 < HP:
                    nslab_q = load_qk_slab(hp + 1)
                    nslab_k = load_qk_slab(CCH + hp + 1)
                    for tg in range(NQ // QG):
                        pending += qk_quantum_steps(hp + 1, nslab_q, tg)
                    for tg in range(N // QG):
                        pending += qk_quantum_steps(CCH + hp + 1, nslab_k, tg)
                if hp in vg1_sched:
                    for tc_i in vg1_sched[hp]:
                        pending += v_quantum_steps(tc_i, 1)
                if hp == 1:
                    for cc in range(CCH):
                        nc.scalar.dma_start(wpT_sb[:, cc, :], wpT_v[cc])

                def pump(k):
                    for _ in range(min(k, len(pending))):
                        pending.pop(0)()

                for qh in range(2):
                    if hp == HP - 1 and qh == 1:
                        # tokens [0,512) have every head's qh0 rows written
                        # (pair 7 normalizes inline at its qh0 boundary), so
                        # the first half of the projection overlaps pair 7;
                        # pumps skip ktp 0 so the proj cc7 matmuls never wait
                        # on the boundary normalize's DVE chain
                        for tc_i in range(4):
                            for og in range(C // QG):
                                pending += proj_steps(tc_i, og)
                    # spread pending over the remaining windows of this pair
                    n_windows = (2 - qh) * KTP
                    per_win = (len(pending) + n_windows - 1) // n_windows
                    av = {
                        ha: ps.tile([VW, QG], F32, tag="ava", name=f"av_{ha}_{qh}"),
                        hb: ps.tile([VW, QG], F32, tag="avb", name=f"av_{hb}_{qh}"),
                    }

                    def scores(h, ktp, name):
                        base = (h % 2) * HD
                        # flat [P, 1024] tile: the exp AP must stay 2D — a
                        # [P, 2, QG] AP costs ACT ~+220ns of per-instruction
                        # decode on every exp
                        st = ps.tile(
                            [P, 2 * QG], F32,
                            tag="sta" if h % 2 == 0 else "stb",
                            name=f"st_{name}",
                        )
                        for i in range(2):
                            kt = 2 * ktp + i
                            nc.tensor.matmul(
                                st[:, i * QG : (i + 1) * QG],
                                lhsT=kT_sb[
                                    base : base + HD, h // 2, kt * P : (kt + 1) * P
                                ],
                                rhs=qT_sb[
                                    base : base + HD, h // 2, qh * QG : (qh + 1) * QG
                                ],
                                start=True,
                                stop=True,
                            )
                        pt = ptpool.tile([P, 2 * QG], BF16, tag="pt", name=f"pt_{name}")
                        nc.scalar.activation(
                            pt[:],
                            st[:],
                            mybir.ActivationFunctionType.Exp,
                            scale=float(SCALE),
                        )
                        return pt

                    def av_mms(h, ktp, pt):
                        for i in range(2):
                            kt = 2 * ktp + i
                            nc.tensor.matmul(
                                av[h][:],
                                lhsT=v_sb[:, kt, h * VW : (h + 1) * VW],
                                rhs=pt[:, i * QG : (i + 1) * QG],
                                start=(ktp == 0 and i == 0),
                                stop=(ktp == KTP - 1 and i == 1),
                            )

                    # window 0: fill the pipe
                    pt_a = scores(ha, 0, f"{ha}_{qh}_0")
                    pt_b = scores(hb, 0, f"{hb}_{qh}_0")

                    for ktp in range(KTP):
                        # deferred normalizes first: pumped steps may include
                        # proj matmuls that read the oT rows these write
                        if ktp == 0:
                            while deferred:
                                normalize(*deferred.pop(0))
                        # pair-0 JIT vg0 tiles, one ktp ahead, split around
                        # the attention matmuls so neither quantum's first
                        # matmul waits on the other's PSUM-ring copy
                        if hp == 0 and qh == 0 and ktp < KTP - 1:
                            run_all(v_quantum_steps(2 * ktp + 2, 0))
                        no_pump = hp == HP - 1 and qh == 1 and ktp == 0
                        # quanta first: they have no waits, so they fill the
                        # PE while this window's exps run; the gated score/AV
                        # matmuls would otherwise head-block the PE FIFO
                        if not no_pump:
                            pump(2)
                        if ktp < KTP - 1:
                            npt_a = scores(ha, ktp + 1, f"{ha}_{qh}_{ktp+1}")
                        av_mms(ha, ktp, pt_a)
                        if hp == 0 and qh == 0 and ktp < KTP - 1:
                            run_all(v_quantum_steps(2 * ktp + 3, 0))
                        elif not no_pump:
                            pump(2)
                        av_mms(hb, ktp, pt_b)
                        if ktp < KTP - 1:
                            npt_b = scores(hb, ktp + 1, f"{hb}_{qh}_{ktp+1}")
                            pt_a, pt_b = npt_a, npt_b
                        if per_win > 4 and not no_pump:
                            pump(per_win - 4)

                    # pair/half boundary: stash unnormalized AV (+ denom row)
                    # so the av PSUM banks recycle for the next half. The
                    # whole last pair normalizes inline: its qh0 rows gate
                    # the projection pumps in qh1, and its qh1 rows gate the
                    # projection tail.
                    last = hp == HP - 1
                    for h in (ha, hb):
                        stash = stashp.tile(
                            [VW, QG], BF16, tag="stash", name=f"stash_{h}_{qh}"
                        )
                        nc.vector.tensor_copy(stash[:], av[h][:])
                        if last:
                            normalize(h, qh, stash)
                        else:
                            deferred.append((h, qh, stash))

                pump(len(pending))

            while deferred:
                normalize(*deferred.pop(0))

            # ---- output projection, second half (tokens [512,1024)) ----
            # the attention score banks are dead by now; rotating the tail
            # groups across sta/stb/q gives a 4-deep psum ring so no group's
            # first matmul waits on a DVE add two groups back
            tail_tags = ["sta", "stb", "q", "q"]
            for gi, (tc_i, og) in enumerate(
                (t, o) for t in range(4, NQ // P) for o in range(C // QG)
            ):
                run_all(proj_steps(tc_i, og, tag=tail_tags[gi % 4]))
    _split_multi_waits(nc)
    return nc


def get_nc():
    if "nc" not in _CACHE:
        _CACHE["nc"] = _build()
    return _CACHE["nc"]


def make_in_maps(x, w_qkv, w_proj, b_proj):
    x = np.asarray(x, np.float32)
    w_qkv = np.asarray(w_qkv, np.float32)
    w_proj = np.asarray(w_proj, np.float32)
    b_proj = np.asarray(b_proj, np.float32)
    # pre-pack w_qk^T as [row-group, c-part, c-chunk, row] so slab DMAs are
    # fully contiguous per partition
    wqkT = np.ascontiguousarray(
        w_qkv[: 2 * C]
        .T.reshape(CCH, P, 2 * CCH, P)
        .transpose(2, 1, 0, 3)
    ).astype(NPBF16)
    wvT = np.ascontiguousarray(w_qkv[2 * C :].T).astype(NPBF16)
    wpT = np.ascontiguousarray(w_proj.T).astype(NPBF16)
    bias = b_proj.reshape(1, C).astype(NPBF16)
    in_maps = []
    for c in range(8):
        b, s = divmod(c, 2)
        xb = x[b].astype(NPBF16)  # [N, C]
        rolled = np.concatenate(
            [xb[s * NQ : (s + 1) * NQ], xb[(1 - s) * NQ : (2 - s) * NQ]], 0
        )
        xT = np.ascontiguousarray(rolled.T)  # [C, N], local queries first
        in_maps.append({"xT": xT, "wqkT": wqkT, "wvT": wvT, "wpT": wpT, "bias": bias})
    return in_maps


def kernel(x, w_qkv, w_proj, b_proj, _res_out=None):
    nc = get_nc()
    in_maps = make_in_maps(x, w_qkv, w_proj, b_proj)
    res = run_bass_kernel_spmd(nc, in_maps, core_ids=list(range(8)))
    if _res_out is not None:
        _res_out.append(res)

    y = np.empty((B, N, C), np.float32)
    for c in range(8):
        b, s = divmod(c, 2)
        y[b, s * NQ : (s + 1) * NQ] = res.results[c]["y"]
    return y


if __name__ == "__main__":
    rng = np.random.default_rng(0)
    inp = {
        "x": rng.standard_normal((B, N, C), dtype=np.float32),
        "w_qkv": rng.standard_normal((3 * C, C), dtype=np.float32) * C**-0.5,
        "w_proj": rng.standard_normal((C, C), dtype=np.float32) * C**-0.5,
        "b_proj": rng.standard_normal(C, dtype=np.float32) * 0.01,
    }
    y = kernel(**inp)
    print("ran", y.shape, y.dtype)
